# revision 1
# baseline (speedup 1.0000x reference)
"""Multi-head causal attention (B=4, T=2048, D=1024, H=16) on 8 TRN2 NeuronCores.

Sharding: data-parallel over batch (4) x tensor-parallel over heads (2 groups
of 8). Core c handles batch c//2, head-group c%2: its Q/K/V projections
(weight-column shards), causal attention for its 8 heads, and a partial
output projection (weight-row shard). The pairwise reduction of the two
partials per batch happens on host (4 cheap bf16 adds).

Datapath is bf16 (inputs converted host-side; PSUM accumulation stays fp32):
matmuls run at full PE rate at any moving size, input DMA halves, DVE mask
multiplies hit the 2x mode. Measured rel err ~4.5e-3 against the fp32
reference (gate 2e-2).

Schedule: chunk-major over (512-query chunk c, head-pair pr) "units" so the
scalar engine's exp stream (the #2 engine load, ~145us) starts ~13us in.
Projection / out-projection work is a (need, pull)-tagged filler list drained
between attention blocks to keep the PE (the bottleneck, ~228us busy) fed;
each round's out-projection is spread across the next round's units. The
causal diagonal runs at 128-query granularity: QK^T moving slices of
512/384/256/128 columns, one strided-AP exp covering just the two heads'
valid strips, AV accumulation windows shrunk to match, and a single shared
128x128 triangular mask applied only to true diagonal blocks. The AV lag
queue (LAG kb's between exp and AV) crosses unit boundaries so a unit's
start interleaves with the previous unit's AV matmuls.

Softmax normalization: denominators ride as a 65th V row through the AV
matmul, reciprocal_approx on DVE, partition-broadcast on the otherwise-idle
Pool engine (GPSIMD cannot touch PSUM, so all psum->sbuf copies stay on
DVE/Act), and the scale is fused into the psum->sbuf attention-output copy.
On the last unit each 128-query block is normalized and out-projected as
soon as its own diagonal AV lands, shortening the endgame drain.
"""

import sys

if "/opt/trn_rl_repo" not in sys.path:
    sys.path.insert(0, "/opt/trn_rl_repo")

import ml_dtypes
import numpy as np

import concourse.bass as bass
import concourse.mybir as mybir
from concourse import bacc
from concourse.bass import MemorySpace
from concourse.tile import TileContext

B, T, D = 4, 2048, 1024
H, DH = 16, 64
HG = 8          # heads per core
GW = HG * DH    # group width = 512
P = 128
KD = D // P     # 8 contraction chunks
NTB = T // P    # 16 key blocks of 128
N_CORES = 8
LAG = 6      # kb's between exp issue and AV consumption
XUNIT = 1    # AV lag queue crosses unit boundaries
PULL = 3     # filler steps pulled forward per kb
PULLOFF = 0  # units of pull-ahead for projection filler

F32 = mybir.dt.float32
BF16 = mybir.dt.bfloat16
U16 = mybir.dt.uint16


def build_nc():
    nc = bacc.Bacc()

    xq = nc.dram_tensor("xq", [D, T], BF16, kind="ExternalInput")
    xk = nc.dram_tensor("xk", [D, T], BF16, kind="ExternalInput")
    xv = nc.dram_tensor("xv", [D, T], BF16, kind="ExternalInput")
    wq = nc.dram_tensor("wq", [D, GW], BF16, kind="ExternalInput")
    wk = nc.dram_tensor("wk", [D, GW], BF16, kind="ExternalInput")
    wv = nc.dram_tensor("wv", [D, GW], BF16, kind="ExternalInput")
    wo = nc.dram_tensor("wo", [GW, D], BF16, kind="ExternalInput")
    msk = nc.dram_tensor("msk", [P, P], BF16, kind="ExternalInput")
    out = nc.dram_tensor("out", [T, D], BF16, kind="ExternalOutput")

    with TileContext(nc) as tc:
        cms = []

        def pool(name, bufs, space=None):
            kw = {"space": space} if space else {}
            cm = tc.tile_pool(name=name, bufs=bufs, **kw)
            cms.append(cm)
            return cm.__enter__()

        big = pool("big", 1)
        ppool = pool("pp", 10)
        xkq = pool("xkq", 8)
        xvp = pool("xvp", 4)
        rtp = pool("rtp", 2)
        bcp = pool("bcp", 2)
        obp = pool("obp", 3)
        sp = pool("sp", 2, MemorySpace.PSUM)    # [128,1024] f32 -> 2 banks x2
        avp = pool("avp", 2, MemorySpace.PSUM)  # [65,512] f32  -> 1 bank x2
        psp = pool("psp", 2, MemorySpace.PSUM)  # [128,512] f32 -> 1 bank x2

        kts = [big.tile([P, T], BF16, name=f"kt{j}") for j in range(4)]
        qts = [big.tile([P, T], BF16, name=f"qt{j}") for j in range(4)]
        # attention output reuses qt storage: qt[:, chunk] is dead after that
        # chunk's QK^T matmuls, exactly when the tail writes it
        aots = qts
        vsb = big.tile([P, NTB, HG * 65], BF16, name="vsb")
        wk_sba = big.tile([P, KD, 256], BF16, name="wk_sba")
        wk_sbb = big.tile([P, KD, 256], BF16, name="wk_sbb")
        wq_sba = big.tile([P, KD, 256], BF16, name="wq_sba")
        wq_sbb = big.tile([P, KD, 256], BF16, name="wq_sbb")
        wv_sb = big.tile([P, KD, GW], BF16, name="wv_sb")
        wo_sb = big.tile([P, 4, D], BF16, name="wo_sb")
        mask_sb = big.tile([P, P], BF16, name="mask_sb")

        vones = vsb.rearrange("p tb (h m) -> p tb h m", h=HG)[:, :, :, 64:65]
        nc.vector.memset(vones.bitcast(U16), 0x3F80)

        lo, hi = slice(0, 64), slice(64, 128)

        xk_t, xq_t, xv_t = {}, {}, {}

        def dma_x(src, store, ch, pool_, tag):
            t = pool_.tile([P, KD, 256], BF16, name=f"x{tag}", tag=tag)
            nc.sync.dma_start(
                t, src.rearrange("(ko p) t -> p ko t", p=P)[:, :, ch * 256:(ch + 1) * 256]
            )
            store[ch] = t

        # ---- upfront DMAs: half-column weight loads sequenced against the
        # x chunks so the PE starts ~4us in and never gaps (a gap re-triggers
        # the slow p-state ramp) ----
        wk_r = wk.rearrange("(ko p) j -> p ko j", p=P)
        wq_r = wq.rearrange("(ko p) j -> p ko j", p=P)
        nc.sync.dma_start(wk_sba, wk_r[:, :, 0:256])
        dma_x(xk, xk_t, 0, xkq, "xk")
        dma_x(xk, xk_t, 1, xkq, "xk")
        nc.sync.dma_start(wk_sbb, wk_r[:, :, 256:GW])
        nc.sync.dma_start(wq_sba, wq_r[:, :, 0:256])
        dma_x(xq, xq_t, 0, xkq, "xq")
        dma_x(xq, xq_t, 1, xkq, "xq")
        nc.sync.dma_start(mask_sb, msk[:, :])
        nc.sync.dma_start(wv_sb, wv.rearrange("(ko p) j -> p ko j", p=P))
        dma_x(xv, xv_t, 0, xvp, "xv")
        dma_x(xv, xv_t, 1, xvp, "xv")
        # wq second half last: its jb1 columns are already in the first half,
        # and jb2/jb3 are not needed until unit 2
        nc.sync.dma_start(wq_sbb, wq_r[:, :, 256:GW])

        # ---- filler steps: (need, pull, kind, fn) ----
        steps = []

        def kq_step(w_halves, xst, dst, ch, jb):
            def fn():
                w_sb = w_halves[jb // 2]
                jo = (jb % 2) * P
                ps = psp.tile([P, 256], F32, name="ps_kq", tag="ps")
                for kd in range(KD):
                    nc.tensor.matmul(
                        ps, w_sb[:, kd, jo:jo + P], xst[ch][:, kd, :],
                        start=(kd == 0), stop=(kd == KD - 1),
                    )
                nc.vector.tensor_copy(dst[jb][:, ch * 256:(ch + 1) * 256], ps)
            return fn

        def v_step(tb):
            def fn():
                xt = xv_t[tb // 2]
                co = (tb % 2) * P
                ps = psp.tile([P, GW], F32, name="ps_v", tag="ps")
                for kd in range(KD):
                    nc.tensor.matmul(
                        ps, xt[:, kd, co:co + P], wv_sb[:, kd, :],
                        start=(kd == 0), stop=(kd == KD - 1),
                    )
                nc.vector.tensor_copy(
                    vsb[:, tb, :].rearrange("p (h m) -> p h m", h=HG)[:, :, 0:64],
                    ps.rearrange("p (h m) -> p h m", h=HG),
                )
            return fn

        def o_step(tb, oc):
            def fn():
                if oc == 0:
                    ob_t[tb % 2] = obp.tile([P, D], BF16, name="ob", tag="ob")
                ob = ob_t[tb % 2]
                ps = psp.tile([P, GW], F32, name="ps_o", tag="ps")
                for jb in range(4):
                    nc.tensor.matmul(
                        ps, aots[jb][:, tb * P:(tb + 1) * P],
                        wo_sb[:, jb, oc * GW:(oc + 1) * GW],
                        start=(jb == 0), stop=(jb == 3),
                    )
                if tb >= 12:
                    nc.scalar.copy(ob[:, oc * GW:(oc + 1) * GW], ps)
                else:
                    nc.vector.tensor_copy(ob[:, oc * GW:(oc + 1) * GW], ps)
                nc.sync.dma_start(
                    out[tb * P:(tb + 1) * P, oc * GW:(oc + 1) * GW],
                    ob[:, oc * GW:(oc + 1) * GW],
                )
            return fn

        ob_t = {}

        def wo_dma():
            nc.sync.dma_start(wo_sb, wo.rearrange("(jb p) o -> p jb o", p=P))

        v_idx, o_idx = {}, {}
        for u in range(16):
            r, pr = u // 4, u % 4
            if pr == 1 and r <= 2:
                # next round's x chunks: listed one round early so lookahead
                # pulls issue the DMAs well before the round boundary
                for ch in (2 * r + 2, 2 * r + 3):
                    steps.append((4 * r + 4, u - 1, "x",
                                  (lambda ch=ch: dma_x(xk, xk_t, ch, xkq, "xk"))))
                    steps.append((4 * r + 4, u - 1, "x",
                                  (lambda ch=ch: dma_x(xq, xq_t, ch, xkq, "xq"))))
                    steps.append((4 * r + 4, u - 1, "x",
                                  (lambda ch=ch: dma_x(xv, xv_t, ch, xvp, "xv"))))
            if u == 3:
                steps.append((u, 0, "x", wo_dma))
            if u == 0:
                # ordered to match DMA arrivals: chunk-0 jb0/jb1 (first wk
                # half), chunk-1 jb0/jb1, then jb2 (second half), then Q jb0
                for ch in (0, 1):
                    for jb in (0, 1):
                        steps.append((0, 0, "p", kq_step((wk_sba, wk_sbb), xk_t, kts, ch, jb)))
                for ch in (0, 1):
                    steps.append((0, 0, "p", kq_step((wk_sba, wk_sbb), xk_t, kts, ch, 2)))
                for ch in (0, 1):
                    steps.append((0, 0, "p", kq_step((wq_sba, wq_sbb), xq_t, qts, ch, 0)))
                for ch in (0, 1):
                    steps.append((1, 0, "p", kq_step((wk_sba, wk_sbb), xk_t, kts, ch, 3)))
            elif u < 4:
                for ch in (2 * r, 2 * r + 1):
                    steps.append((u, u - PULLOFF, "p",
                                  kq_step((wq_sba, wq_sbb), xq_t, qts, ch, pr)))
            else:
                # Q steps first: with diagonal-first kb order the unit's
                # opening QK reads the new kt chunk, so the K copies overlap
                # the remaining forced steps instead of gating the unit
                for ch in (2 * r, 2 * r + 1):
                    steps.append((u, u - PULLOFF, "p",
                                  kq_step((wq_sba, wq_sbb), xq_t, qts, ch, pr)))
                for ch in (2 * r, 2 * r + 1):
                    steps.append((u, u - PULLOFF, "p",
                                  kq_step((wk_sba, wk_sbb), xk_t, kts, ch, pr)))
            if pr == 0:
                for tb in range(4 * r, 4 * r + 4):
                    v_idx[tb] = len(steps)
                    steps.append((17, u - PULLOFF, "p", v_step(tb)))
            if pr in (1, 2, 3) and r >= 1:
                tbs = list(range(4 * (r - 1), 4 * r))
                grp = ({1: tbs[0:1], 2: tbs[1:2], 3: tbs[2:4]}[pr])
                for tb in grp:
                    for oc in (0, 1):
                        steps.append((u, u, "o", o_step(tb, oc)))
        for tb in range(12, 16):
            for oc in (0, 1):
                o_idx[(tb, oc)] = len(steps)
                steps.append((18, 18, "o", o_step(tb, oc)))

        emitted = [False] * len(steps)
        head = [0]

        def emit_step(i):
            if not emitted[i]:
                emitted[i] = True
                steps[i][3]()

        def ensure_v(tb):
            for t in range(tb + 1):
                emit_step(v_idx[t])

        def drain_force(maxneed):
            while head[0] < len(steps) and emitted[head[0]]:
                head[0] += 1
            i = head[0]
            while i < len(steps):
                if not emitted[i] and steps[i][0] <= maxneed:
                    emitted[i] = True
                    steps[i][3]()
                elif not emitted[i] and steps[i][0] > maxneed + 4:
                    break
                i += 1

        def drain_pull(u, limit):
            while head[0] < len(steps) and emitted[head[0]]:
                head[0] += 1
            n, i = 0, head[0]
            scanned = 0
            while i < len(steps) and n < limit and scanned < 80:
                if not emitted[i] and steps[i][1] <= u:
                    emitted[i] = True
                    steps[i][3]()
                    n += 1
                scanned += 1
                i += 1

        # ---- attention units, chunk-major; AV lag queue crosses units ----
        scale = float(DH) ** -0.5
        pend = []       # (unit, kb, F, d0, pp)
        ctx = {}        # unit -> dict(av1, av2, pr, nblk, cs0)

        def emit_av(e):
            uu, kb, F, d0, pp, st, sop = e
            cx = ctx[uu]
            ensure_v(kb)
            pr_ = cx["pr"]
            nc.tensor.matmul(
                cx["av1"][:, d0:d0 + F],
                vsb[:, kb, (2 * pr_) * 65:(2 * pr_) * 65 + 65],
                pp[:, 0:F], start=st, stop=sop, skip_group_check=True,
            )
            nc.tensor.matmul(
                cx["av2"][:, d0:d0 + F],
                vsb[:, kb, (2 * pr_ + 1) * 65:(2 * pr_ + 1) * 65 + 65],
                pp[:, 512:512 + F], start=st, stop=sop, skip_group_check=True,
            )
            if uu == 15:
                # column block qb is final once its diagonal AV (j == qb)
                # lands: normalize + out-project it while later AVs run
                j = kb - 4 * 3
                if j >= 0:
                    emit_tail15_qb(cx, j)
            cx["left"] -= 1
            if cx["left"] == 0:
                emit_tail(uu)

        def emit_tail15_qb(cx, q4):
            av1, av2, aot, cs0 = cx["av1"], cx["av2"], cx["aot"], cx["cs0"]
            qs4 = slice(q4 * 128, q4 * 128 + 128)
            cs4 = slice(cs0 + q4 * 128, cs0 + q4 * 128 + 128)
            rt4 = rtp.tile([1, 256], F32, name="rt4", tag="rt4")
            nc.vector.tensor_copy(rt4[0:1, 0:128], av1[64:65, qs4])
            nc.vector.tensor_copy(rt4[0:1, 128:256], av2[64:65, qs4])
            nc.vector.reciprocal_approx_fast(rt4, rt4)
            bc4 = bcp.tile([P, 256], F32, name="bc4", tag="bc4")
            nc.gpsimd.partition_broadcast(bc4[:, 0:128], rt4[0:1, 0:128])
            nc.gpsimd.partition_broadcast(bc4[:, 128:256], rt4[0:1, 128:256])
            nc.vector.tensor_mul(aot[lo, cs4], av1[0:64, qs4], bc4[0:64, 0:128])
            nc.vector.tensor_mul(aot[hi, cs4], av2[0:64, qs4], bc4[64:128, 128:256])
            emit_step(o_idx[(12 + q4, 0)])
            emit_step(o_idx[(12 + q4, 1)])

        def emit_tail(uu):
            cx = ctx.pop(uu)
            av1, av2, aot, cs0 = cx["av1"], cx["av2"], cx["aot"], cx["cs0"]
            if uu < 15:
                rt = rtp.tile([1, 1024], F32, name="rt", tag="rt")
                nc.vector.tensor_copy(rt[0:1, 0:512], av1[64:65, :])
                nc.vector.tensor_copy(rt[0:1, 512:1024], av2[64:65, :])
                nc.vector.reciprocal_approx_fast(rt, rt)
                bc = bcp.tile([P, 1024], F32, name="bc", tag="bc")
                nc.gpsimd.partition_broadcast(bc[:, 0:512], rt[0:1, 0:512])
                nc.gpsimd.partition_broadcast(bc[:, 512:1024], rt[0:1, 512:1024])
                cs = slice(cs0, cs0 + 512)
                nc.vector.tensor_mul(aot[lo, cs], av1[0:64, :], bc[0:64, 0:512])
                nc.vector.tensor_mul(aot[hi, cs], av2[0:64, :], bc[64:128, 512:1024])
            else:
                pass  # handled per-qb in emit_tail15_qb

        for u in range(16):
            c, pr = u // 4, u % 4
            kt, qt, aot = kts[pr], qts[pr], aots[pr]
            nblk = 4 * (c + 1)
            cs0 = 512 * c
            drain_force(u)

            av1 = avp.tile([65, 512], F32, name="av1", tag="av")
            av2 = avp.tile([65, 512], F32, name="av2", tag="av")
            ctx[u] = {"av1": av1, "av2": av2, "pr": pr, "nblk": nblk,
                      "cs0": cs0, "aot": aot, "left": nblk}

            if u < 15:
                kb_order = list(range(4 * c, nblk)) + list(range(0, 4 * c))
            else:
                kb_order = list(range(nblk))
            for ki, kb in enumerate(kb_order):
                j = kb - 4 * c
                F = 512 if j < 0 else 512 - 128 * j
                d0 = 0 if j < 0 else 128 * j
                s_pair = sp.tile([P, 1024], F32, name="s_pair", tag="sp")
                ks = slice(kb * P, (kb + 1) * P)
                qs = slice(cs0 + d0, cs0 + d0 + F)
                nc.tensor.matmul(
                    s_pair[:, 0:F], kt[lo, ks], qt[lo, qs], start=True, stop=True,
                )
                nc.tensor.matmul(
                    s_pair[:, 512:512 + F], kt[hi, ks], qt[hi, qs],
                    start=True, stop=True,
                )
                pp = ppool.tile([P, 1024], BF16, name="p_pair", tag="pp")
                sv = s_pair.rearrange("p (h q) -> p h q", h=2)[:, :, 0:F]
                pv = pp.rearrange("p (h q) -> p h q", h=2)[:, :, 0:F]
                nc.scalar.activation(
                    pv, sv, mybir.ActivationFunctionType.Exp, scale=scale,
                )
                if j >= 0:
                    # mask the 128x128 diagonal block (first 128 cols of strip)
                    nc.vector.tensor_mul(pp[:, 0:128], pp[:, 0:128], mask_sb)
                    nc.vector.tensor_mul(pp[:, 512:640], pp[:, 512:640], mask_sb)
                drain_pull(u, PULL)
                pend.append((u, kb, F, d0, pp, ki == 0, ki == nblk - 1))
                lag_u = LAG if u < 15 else 3
                if len(pend) > lag_u:
                    emit_av(pend.pop(0))
                if u == 15 and kb >= 12:
                    # drain the lag queue faster at the very end so the
                    # per-qb tails + final out-projection start earlier
                    for _ in range(2):
                        if pend:
                            emit_av(pend.pop(0))
            if not XUNIT:
                while pend:
                    emit_av(pend.pop(0))
                    drain_pull(u, 1)

        while pend:
            emit_av(pend.pop(0))
            drain_pull(16, 1)
        drain_force(99)

        for cm in reversed(cms):
            cm.__exit__(None, None, None)

    nc.finalize()
    return nc


def _to_bf16(a):
    return np.ascontiguousarray(a).astype(ml_dtypes.bfloat16)


def make_in_maps(q, k, v, Wq, Wk, Wv, Wo):
    mask_bf = (
        np.arange(P)[None, :] >= np.arange(P)[:, None]
    ).astype(ml_dtypes.bfloat16)
    xs = [
        {"xq": _to_bf16(q[b].T), "xk": _to_bf16(k[b].T), "xv": _to_bf16(v[b].T)}
        for b in range(B)
    ]
    ws = []
    for g in range(2):
        hs = slice(g * GW, (g + 1) * GW)
        ws.append({
            "wq": _to_bf16(Wq[hs, :].T),
            "wk": _to_bf16(Wk[hs, :].T),
            "wv": _to_bf16(Wv[hs, :].T),
            "wo": _to_bf16(Wo[:, hs].T),
        })
    return [
        {**xs[c // 2], **ws[c % 2], "msk": mask_bf} for c in range(N_CORES)
    ]


_NC_CACHE = None


def kernel(q, k, v, mask, Wq, Wk, Wv, Wo):
    global _NC_CACHE
    if _NC_CACHE is None:
        _NC_CACHE = build_nc()
    nc = _NC_CACHE

    from concourse.bass_utils import run_bass_kernel_spmd

    q, k, v = np.asarray(q), np.asarray(k), np.asarray(v)
    Wq, Wk, Wv, Wo = (np.asarray(t) for t in (Wq, Wk, Wv, Wo))
    in_maps = make_in_maps(q, k, v, Wq, Wk, Wv, Wo)

    r = run_bass_kernel_spmd(nc, in_maps, core_ids=list(range(N_CORES)))
    parts = [np.asarray(r.results[c]["out"], dtype=np.float32) for c in range(N_CORES)]
    y = np.stack([parts[2 * b] + parts[2 * b + 1] for b in range(B)])
    return y



# revision 4
# speedup vs baseline: 1.0130x; 1.0130x over previous
"""Multi-head causal attention (B=4, T=2048, D=1024, H=16) on 8 TRN2 NeuronCores.

Sharding: data-parallel over batch (4) x tensor-parallel over heads (2 groups
of 8). Core c handles batch c//2, head-group c%2. Partial out-projections are
pairwise-summed on host.

Datapath (v2):
- Q/K/V projections run as 3-term fp8 residual matmuls in DoubleRow mode
  (x and W shipped from host as e4m3 hi+lo splits, W prescaled by 32):
  (Wh+Wl).T xh + Wh.T xl per kd-pair, 0.75x the bf16 row count at ~bf16
  accuracy. 12 DoubleRow matmuls per [128-out, 256-token] tile.
- K is stored as single-level fp8 (the one budgeted quantization, damped by
  small score magnitudes); Q is stored as an fp8 hi+lo pair. QK^T then runs
  in DoubleRow: lhsT = K dup'd via a stride-0 ktile dim, rhs = (q_hi, q_lo),
  0.5F cycles per (key-block, head) - half the bf16 cost, Q effectively exact.
- exp on Act (bf16 probs), triangular-mask multiplies on Pool.
- AV runs *swapped*: probs block [128 keys, 128 queries] stationary, V
  [128 keys, 64] + ones column [128, 1] moving, psum accumulates
  [query, feat] over key blocks: 65 moving rows per (key block, query block,
  head) vs 128 in the probs-moving orientation. Denominators land as
  per-query-partition scalars: one reciprocal_approx over [128, 8] and one
  stride-0-broadcast tensor_tensor multiply drain per unit replace the whole
  row-copy/reciprocal/partition-broadcast pipeline of the bf16 baseline.
- The drained attention output [query, feat] is DMA-transposed (128x128
  tiles) into [feat, token] for a bf16 out-projection (baseline o_step).

Cost model: PE ~353k cycles (~147us), Act ~144us (exp only), DVE ~85us
(all psum drains + recip), Pool ~15us (masks). Measured rel err ~1.4e-2
in the numpy model of this exact datapath (gate 2e-2).
"""

import sys

if "/opt/trn_rl_repo" not in sys.path:
    sys.path.insert(0, "/opt/trn_rl_repo")

import ml_dtypes
import numpy as np

import concourse.bass as bass
import concourse.mybir as mybir
from concourse import bacc
from concourse.bass import MemorySpace
from concourse.tile import TileContext

B, T, D = 4, 2048, 1024
H, DH = 16, 64
HG = 8          # heads per core
GW = HG * DH    # group width = 512
P = 128
KD = D // P     # 8 contraction chunks
NTB = T // P    # 16 key blocks of 128
N_CORES = 8
LAG = 6      # kb's between exp issue and AV consumption
PULL = 3     # filler steps pulled forward per kb
WS = 32.0    # host-side weight prescale for fp8 (subnormal avoidance)

F32 = mybir.dt.float32
BF16 = mybir.dt.bfloat16
FP8 = mybir.dt.float8e4
U16 = mybir.dt.uint16
DR = mybir.MatmulPerfMode.DoubleRow


def build_nc():
    nc = bacc.Bacc()

    xs_in = {}
    for nm in ("xq", "xk", "xv"):
        for lv in ("h", "l"):
            xs_in[nm + lv] = nc.dram_tensor(nm + lv, [D, T], FP8, kind="ExternalInput")
    ws_in = {}
    for nm in ("wq", "wk", "wv"):
        for lv in ("h", "l"):
            ws_in[nm + lv] = nc.dram_tensor(nm + lv, [D, GW], FP8, kind="ExternalInput")
    wo = nc.dram_tensor("wo", [GW, D], BF16, kind="ExternalInput")
    msk = nc.dram_tensor("msk", [P, P], BF16, kind="ExternalInput")
    out = nc.dram_tensor("out", [T, D], BF16, kind="ExternalOutput")

    with TileContext(nc) as tc:
        cms = []

        def pool(name, bufs, space=None):
            kw = {"space": space} if space else {}
            cm = tc.tile_pool(name=name, bufs=bufs, **kw)
            cms.append(cm)
            return cm.__enter__()

        big = pool("big", 1)
        ppool = pool("pp", 8)
        xkq = pool("xkq", 16)
        xvp = pool("xvp", 8)
        aop = pool("aop", 2)
        obp = pool("obp", 3)
        sp = pool("sp", 2, MemorySpace.PSUM)    # [128,1024] f32 -> 2 banks x2
        avp = pool("avp", 1, MemorySpace.PSUM)  # av 1 bank + dn 1 bank
        psp = pool("psp", 2, MemorySpace.PSUM)  # [128,512] f32 -> 1 bank x2

        kts = [big.tile([P, T], FP8, name=f"kt{j}") for j in range(4)]
        qts = [big.tile([P, 2, T], FP8, name=f"qt{j}") for j in range(4)]
        vsb = big.tile([P, NTB, HG * 65], BF16, name="vsb")
        wsb = {}
        for nm in ("wq", "wk", "wv"):
            for lv in ("h", "l"):
                wsb[nm + lv] = big.tile([P, KD, GW], FP8, name=f"{nm}{lv}_sb")
        wo_sb = big.tile([P, 4, D], BF16, name="wo_sb")
        mask_sb = big.tile([P, P], BF16, name="mask_sb")
        aotT = [big.tile([P, 4, 512], BF16, name=f"aotT{c}") for c in range(4)]

        vones = vsb.rearrange("p tb (h m) -> p tb h m", h=HG)[:, :, :, 64:65]
        nc.vector.memset(vones.bitcast(U16), 0x3F80)

        lo, hi = slice(0, 64), slice(64, 128)

        xk_t, xq_t, xv_t = {}, {}, {}

        def dma_x(nm, store, ch, pool_, tag):
            th = pool_.tile([P, KD, 256], FP8, name=f"x{tag}h", tag=tag)
            tl = pool_.tile([P, KD, 256], FP8, name=f"x{tag}l", tag=tag)
            for t, lv in ((th, "h"), (tl, "l")):
                nc.sync.dma_start(
                    t,
                    xs_in[nm + lv].rearrange("(ko p) t -> p ko t", p=P)[
                        :, :, ch * 256:(ch + 1) * 256
                    ],
                )
            store[ch] = (th, tl)

        def dma_w(nm, lv, j0, j1):
            nc.sync.dma_start(
                wsb[nm + lv][:, :, j0:j1],
                ws_in[nm + lv].rearrange("(ko p) j -> p ko j", p=P)[:, :, j0:j1],
            )

        dma_w("wk", "h", 0, 256)
        dma_x("xk", xk_t, 0, xkq, "xk")
        dma_x("xk", xk_t, 1, xkq, "xk")
        dma_w("wk", "l", 0, 256)
        dma_w("wk", "h", 256, GW)
        dma_w("wk", "l", 256, GW)
        dma_w("wq", "h", 0, 256)
        dma_x("xq", xq_t, 0, xkq, "xq")
        dma_x("xq", xq_t, 1, xkq, "xq")
        dma_w("wq", "l", 0, 256)
        nc.sync.dma_start(mask_sb, msk[:, :])
        dma_w("wv", "h", 0, GW)
        dma_w("wv", "l", 0, GW)
        dma_x("xv", xv_t, 0, xvp, "xv")
        dma_x("xv", xv_t, 1, xvp, "xv")
        dma_w("wq", "h", 256, GW)
        dma_w("wq", "l", 256, GW)

        # ---- filler steps: (need, pull, kind, fn) ----
        steps = []

        def kq_step(wh_sb, wl_sb, xst, dst, ch, jb, is_q):
            def fn():
                xht, xlt = xst[ch]
                pst = psp.tile([P, GW], F32, name="ps_kq", tag="ps")
                ps = pst[:, 0:256]
                n = 0
                for kdp in range(4):
                    kk = slice(2 * kdp, 2 * kdp + 2)
                    for wsl, xsl in (
                        (wh_sb[:, kk, jb * P:(jb + 1) * P], xht[:, kk, :]),
                        (wl_sb[:, kk, jb * P:(jb + 1) * P], xht[:, kk, :]),
                        (wh_sb[:, kk, jb * P:(jb + 1) * P], xlt[:, kk, :]),
                    ):
                        nc.tensor.matmul(
                            ps, wsl, xsl, start=(n == 0), stop=(n == 11),
                            perf_mode=DR, skip_group_check=True,
                        )
                        n += 1
                cs = slice(ch * 256, (ch + 1) * 256)
                if is_q:
                    nc.vector.tensor_copy(dst[jb][:, 0, cs], ps)
                    nc.vector.scalar_tensor_tensor(
                        out=dst[jb][:, 1, cs], in0=ps, scalar=1.0,
                        in1=dst[jb][:, 0, cs],
                        op0=mybir.AluOpType.mult, op1=mybir.AluOpType.subtract,
                    )
                else:
                    nc.vector.tensor_copy(dst[jb][:, cs], ps)
            return fn

        def v_step(tb):
            def fn():
                xht, xlt = xv_t[tb // 2]
                co = (tb % 2) * P
                ps = psp.tile([P, GW], F32, name="ps_v", tag="ps")
                n = 0
                for kdp in range(4):
                    kk = slice(2 * kdp, 2 * kdp + 2)
                    for xsl, wsl in (
                        (xht[:, kk, co:co + P], wsb["wvh"][:, kk, :]),
                        (xlt[:, kk, co:co + P], wsb["wvh"][:, kk, :]),
                        (xht[:, kk, co:co + P], wsb["wvl"][:, kk, :]),
                    ):
                        nc.tensor.matmul(
                            ps, xsl, wsl, start=(n == 0), stop=(n == 11),
                            perf_mode=DR, skip_group_check=True,
                        )
                        n += 1
                nc.vector.tensor_copy(
                    vsb[:, tb, :].rearrange("p (h m) -> p h m", h=HG)[:, :, 0:64],
                    ps.rearrange("p (h m) -> p h m", h=HG),
                )
            return fn

        def o_step(tb, oc):
            def fn():
                if oc == 0:
                    ob_t[tb % 2] = obp.tile([P, D], BF16, name="ob", tag="ob")
                ob = ob_t[tb % 2]
                c = tb // 4
                ps = psp.tile([P, GW], F32, name="ps_o", tag="ps")
                for jb in range(4):
                    nc.tensor.matmul(
                        ps, aotT[c][:, jb, (tb % 4) * P:(tb % 4 + 1) * P],
                        wo_sb[:, jb, oc * GW:(oc + 1) * GW],
                        start=(jb == 0), stop=(jb == 3),
                    )
                # 1/WS undoes the host-side V-weight prescale
                if tb >= 12:
                    nc.scalar.mul(ob[:, oc * GW:(oc + 1) * GW], ps, 1.0 / WS)
                else:
                    nc.vector.tensor_scalar(
                        out=ob[:, oc * GW:(oc + 1) * GW], in0=ps,
                        scalar1=1.0 / WS, scalar2=None, op0=mybir.AluOpType.mult,
                    )
                nc.sync.dma_start(
                    out[tb * P:(tb + 1) * P, oc * GW:(oc + 1) * GW],
                    ob[:, oc * GW:(oc + 1) * GW],
                )
            return fn

        ob_t = {}

        def wo_dma():
            nc.sync.dma_start(wo_sb, wo.rearrange("(jb p) o -> p jb o", p=P))

        v_idx, o_idx = {}, {}
        for u in range(16):
            r, pr = u // 4, u % 4
            if pr == 1 and r <= 2:
                for ch in (2 * r + 2, 2 * r + 3):
                    steps.append((4 * r + 4, u - 1, "x",
                                  (lambda ch=ch: dma_x("xk", xk_t, ch, xkq, "xk"))))
                    steps.append((4 * r + 4, u - 1, "x",
                                  (lambda ch=ch: dma_x("xq", xq_t, ch, xkq, "xq"))))
                    steps.append((4 * r + 4, u - 1, "x",
                                  (lambda ch=ch: dma_x("xv", xv_t, ch, xvp, "xv"))))
            if u == 3:
                steps.append((u, 0, "x", wo_dma))
            if u == 0:
                for ch in (0, 1):
                    for jb in (0, 1):
                        steps.append((0, 0, "p", kq_step(wsb["wkh"], wsb["wkl"], xk_t, kts, ch, jb, False)))
                for ch in (0, 1):
                    steps.append((0, 0, "p", kq_step(wsb["wkh"], wsb["wkl"], xk_t, kts, ch, 2, False)))
                for ch in (0, 1):
                    steps.append((0, 0, "p", kq_step(wsb["wqh"], wsb["wql"], xq_t, qts, ch, 0, True)))
                for ch in (0, 1):
                    steps.append((1, 0, "p", kq_step(wsb["wkh"], wsb["wkl"], xk_t, kts, ch, 3, False)))
            elif u < 4:
                for ch in (2 * r, 2 * r + 1):
                    steps.append((u, u, "p",
                                  kq_step(wsb["wqh"], wsb["wql"], xq_t, qts, ch, pr, True)))
            else:
                for ch in (2 * r, 2 * r + 1):
                    steps.append((u, u, "p",
                                  kq_step(wsb["wqh"], wsb["wql"], xq_t, qts, ch, pr, True)))
                for ch in (2 * r, 2 * r + 1):
                    steps.append((u, u, "p",
                                  kq_step(wsb["wkh"], wsb["wkl"], xk_t, kts, ch, pr, False)))
            if pr == 0:
                for tb in range(4 * r, 4 * r + 4):
                    v_idx[tb] = len(steps)
                    steps.append((17, u, "p", v_step(tb)))
            if pr in (1, 2, 3) and r >= 1:
                tbs = list(range(4 * (r - 1), 4 * r))
                grp = ({1: tbs[0:1], 2: tbs[1:2], 3: tbs[2:4]}[pr])
                for tb in grp:
                    for oc in (0, 1):
                        steps.append((u, u, "o", o_step(tb, oc)))
        for tb in range(12, 16):
            for oc in (0, 1):
                o_idx[(tb, oc)] = len(steps)
                steps.append((18, 18, "o", o_step(tb, oc)))

        emitted = [False] * len(steps)
        head = [0]

        def emit_step(i):
            if not emitted[i]:
                emitted[i] = True
                steps[i][3]()

        def ensure_v(tb):
            for t in range(tb + 1):
                emit_step(v_idx[t])

        def drain_force(maxneed):
            while head[0] < len(steps) and emitted[head[0]]:
                head[0] += 1
            i = head[0]
            while i < len(steps):
                if not emitted[i] and steps[i][0] <= maxneed:
                    emitted[i] = True
                    steps[i][3]()
                elif not emitted[i] and steps[i][0] > maxneed + 4:
                    break
                i += 1

        def drain_pull(u, limit):
            while head[0] < len(steps) and emitted[head[0]]:
                head[0] += 1
            n, i = 0, head[0]
            scanned = 0
            while i < len(steps) and n < limit and scanned < 80:
                if not emitted[i] and steps[i][1] <= u:
                    emitted[i] = True
                    steps[i][3]()
                    n += 1
                scanned += 1
                i += 1

        # ---- attention units, chunk-major; AV lag queue crosses units ----
        scale = float(DH) ** -0.5 / (WS * WS)
        pend = []       # (unit, kb, F, d0, pp)
        ctx = {}        # unit -> state

        def emit_av(e):
            uu, kb, F, d0, pp = e
            cx = ctx[uu]
            ensure_v(kb)
            if cx["av"] is None:
                cx["av"] = avp.tile([P, 8, 64], F32, name="av", tag="av")
                cx["dn"] = avp.tile([P, 8], F32, name="dn", tag="dn")
            av, dn, c, pr = cx["av"], cx["dn"], cx["c"], cx["pr"]
            j = kb - 4 * c
            ppv = pp.rearrange("p (h q) -> p h q", h=2)
            for qb in range(max(j, 0), 4):
                last = (kb == 4 * c + qb) if cx["diag_last"] else (kb == cx["last"][qb])
                for h in range(2):
                    lhs = ppv[:, h, qb * P - d0:(qb + 1) * P - d0]
                    vcol = (2 * pr + h) * 65
                    nc.tensor.matmul(
                        av[:, 2 * qb + h, :], lhs, vsb[:, kb, vcol:vcol + 64],
                        start=not cx["started"], stop=last, skip_group_check=True,
                    )
                    cx["started"] = True
                    nc.tensor.matmul(
                        dn[:, 2 * qb + h:2 * qb + h + 1], lhs,
                        vsb[:, kb, vcol + 64:vcol + 65],
                        start=not cx["dn_started"], stop=last, skip_group_check=True,
                    )
                    cx["dn_started"] = True
            cx["left"] -= 1
            if cx["diag_last"] and j >= 0:
                emit_tail_qb(cx, j)
            if cx["left"] == 0:
                if not cx["diag_last"]:
                    emit_tail(uu)
                ctx.pop(uu)

        def alloc_tail(cx):
            if cx["rcp"] is None:
                cx["rcp"] = aop.tile([P, 4, 2, 1], F32, name="rcp", tag="rcp")
                cx["ao"] = aop.tile([P, 4, 2, 64], BF16, name="ao", tag="ao")

        def emit_tail_qb(cx, qb):
            # u=15 path: query block qb is final once its diagonal AV lands
            alloc_tail(cx)
            av, dn, rcp, ao = cx["av"], cx["dn"], cx["rcp"], cx["ao"]
            c, pr = cx["c"], cx["pr"]
            nc.vector.reciprocal_approx_fast(
                rcp[:, qb, :, :],
                dn[:, 2 * qb:2 * qb + 2].rearrange("p (n o) -> p n o", o=1),
            )
            nc.vector.tensor_tensor(
                out=ao[:, qb, :, :],
                in0=av[:, 2 * qb:2 * qb + 2, :].rearrange("p (a b) f -> p a b f", b=2)[:, 0],
                in1=rcp[:, qb, :, :].broadcast_to([P, 2, 64]),
                op=mybir.AluOpType.mult,
            )
            nc.sync.dma_start_transpose(
                aotT[c][:, pr, qb * P:(qb + 1) * P],
                ao[:, qb, :, :].rearrange("p a b -> p (a b)"),
            )
            emit_step(o_idx[(12 + qb, 0)])
            emit_step(o_idx[(12 + qb, 1)])

        def emit_tail(uu):
            cx = ctx[uu]
            alloc_tail(cx)
            av, dn, rcp, ao = cx["av"], cx["dn"], cx["rcp"], cx["ao"]
            c, pr = cx["c"], cx["pr"]
            nc.vector.reciprocal_approx_fast(
                rcp.rearrange("p a b o -> p (a b) o"),
                dn.rearrange("p (n o) -> p n o", o=1),
            )
            nc.vector.tensor_tensor(
                out=ao,
                in0=av.rearrange("p (a b) f -> p a b f", b=2),
                in1=rcp.broadcast_to([P, 4, 2, 64]),
                op=mybir.AluOpType.mult,
            )
            for qb in range(4):
                nc.sync.dma_start_transpose(
                    aotT[c][:, pr, qb * P:(qb + 1) * P],
                    ao[:, qb, :, :].rearrange("p a b -> p (a b)"),
                )

        for u in range(16):
            c, pr = u // 4, u % 4
            kt, qt = kts[pr], qts[pr]
            nblk = 4 * (c + 1)
            drain_force(u)

            if u < 15:
                kb_order = list(range(4 * c, nblk)) + list(range(0, 4 * c))
            else:
                kb_order = list(range(nblk))
            last = {}
            for kb in kb_order:
                j = kb - 4 * c
                for qb in range(max(j, 0), 4):
                    last[qb] = kb
            ctx[u] = {"av": None, "dn": None, "rcp": None, "ao": None,
                      "pr": pr, "c": c, "u": u, "left": nblk, "last": last,
                      "started": False, "dn_started": False,
                      "diag_last": u == 15}

            for ki, kb in enumerate(kb_order):
                j = kb - 4 * c
                F = 512 if j < 0 else 512 - 128 * j
                d0 = 0 if j < 0 else 128 * j
                s_pair = sp.tile([P, 1024], F32, name="s_pair", tag="sp")
                ks = slice(kb * P, (kb + 1) * P)
                qs = slice(512 * c + d0, 512 * c + d0 + F)
                kt_lo = kt[lo, ks].rearrange("p (o f) -> p o f", o=1).broadcast_to([64, 2, P])
                kt_hi = kt[hi, ks].rearrange("p (o f) -> p o f", o=1).broadcast_to([64, 2, P])
                nc.tensor.matmul(
                    s_pair[:, 0:F], kt_lo, qt[lo, :, qs], start=True, stop=True,
                    perf_mode=DR, skip_group_check=True,
                )
                nc.tensor.matmul(
                    s_pair[:, 512:512 + F], kt_hi, qt[hi, :, qs],
                    start=True, stop=True, perf_mode=DR, skip_group_check=True,
                )
                pp = ppool.tile([P, 1024], BF16, name="p_pair", tag="pp")
                sv = s_pair.rearrange("p (h q) -> p h q", h=2)[:, :, 0:F]
                pv = pp.rearrange("p (h q) -> p h q", h=2)[:, :, 0:F]
                nc.scalar.activation(
                    pv, sv, mybir.ActivationFunctionType.Exp, scale=scale,
                )
                if j >= 0:
                    # triangular mask on the 128x128 diagonal block (Pool)
                    ppm = pp.rearrange("p (h q) -> p h q", h=2)[:, :, 0:128]
                    nc.gpsimd.tensor_tensor(
                        out=ppm, in0=ppm,
                        in1=mask_sb.rearrange("p (o f) -> p o f", o=1).broadcast_to([P, 2, P]),
                        op=mybir.AluOpType.mult,
                    )
                drain_pull(u, PULL)
                pend.append((u, kb, F, d0, pp))
                lag_u = LAG if u < 15 else 3
                if len(pend) > lag_u:
                    emit_av(pend.pop(0))
                if u == 15 and kb >= 12:
                    for _ in range(2):
                        if pend:
                            emit_av(pend.pop(0))

        while pend:
            emit_av(pend.pop(0))
            drain_pull(16, 1)
        drain_force(99)

        for cm in reversed(cms):
            cm.__exit__(None, None, None)

    nc.finalize()
    return nc


def _split8(a):
    hi = np.asarray(a, np.float32).astype(ml_dtypes.float8_e4m3)
    lo = (np.asarray(a, np.float32) - hi.astype(np.float32)).astype(
        ml_dtypes.float8_e4m3)
    return hi, lo


def make_in_maps(q, k, v, Wq, Wk, Wv, Wo):
    mask_bf = (
        np.arange(P)[None, :] >= np.arange(P)[:, None]
    ).astype(ml_dtypes.bfloat16)
    xs = []
    for b in range(B):
        d = {}
        for nm, t in (("xq", q), ("xk", k), ("xv", v)):
            h, l = _split8(np.ascontiguousarray(t[b].T))
            d[nm + "h"], d[nm + "l"] = h, l
        xs.append(d)
    ws = []
    for g in range(2):
        hs = slice(g * GW, (g + 1) * GW)
        d = {}
        for nm, W in (("wq", Wq), ("wk", Wk), ("wv", Wv)):
            h, l = _split8(np.ascontiguousarray(W[hs, :].T) * WS)
            d[nm + "h"], d[nm + "l"] = h, l
        d["wo"] = np.ascontiguousarray(Wo[:, hs].T).astype(ml_dtypes.bfloat16)
        ws.append(d)
    return [
        {**xs[c // 2], **ws[c % 2], "msk": mask_bf} for c in range(N_CORES)
    ]


_NC_CACHE = None


def kernel(q, k, v, mask, Wq, Wk, Wv, Wo):
    global _NC_CACHE
    if _NC_CACHE is None:
        _NC_CACHE = build_nc()
    nc = _NC_CACHE

    from concourse.bass_utils import run_bass_kernel_spmd

    q, k, v = np.asarray(q), np.asarray(k), np.asarray(v)
    Wq, Wk, Wv, Wo = (np.asarray(t) for t in (Wq, Wk, Wv, Wo))
    in_maps = make_in_maps(q, k, v, Wq, Wk, Wv, Wo)

    r = run_bass_kernel_spmd(nc, in_maps, core_ids=list(range(N_CORES)))
    parts = [np.asarray(r.results[c]["out"], dtype=np.float32) for c in range(N_CORES)]
    y = np.stack([parts[2 * b] + parts[2 * b + 1] for b in range(B)])
    return y


# revision 5
# speedup vs baseline: 1.1561x; 1.1413x over previous
"""Multi-head causal attention (B=4, T=2048, D=1024, H=16) on 8 TRN2 NeuronCores.

Sharding: data-parallel over batch (4) x tensor-parallel over heads (2 groups
of 8). Core c handles batch c//2, head-group c%2. Partial out-projections are
pairwise-summed on host.

Datapath (v2):
- Q/K/V projections run as 3-term fp8 residual matmuls in DoubleRow mode
  (x and W shipped from host as e4m3 hi+lo splits, W prescaled by 32):
  (Wh+Wl).T xh + Wh.T xl per kd-pair, 0.75x the bf16 row count at ~bf16
  accuracy. 12 DoubleRow matmuls per [128-out, 256-token] tile.
- K is stored as single-level fp8 (the one budgeted quantization, damped by
  small score magnitudes); Q is stored as an fp8 hi+lo pair. QK^T then runs
  in DoubleRow: lhsT = K dup'd via a stride-0 ktile dim, rhs = (q_hi, q_lo),
  0.5F cycles per (key-block, head) - half the bf16 cost, Q effectively exact.
- exp on Act (bf16 probs), triangular-mask multiplies on Pool.
- AV runs *swapped*: probs block [128 keys, 128 queries] stationary, V
  [128 keys, 64] + ones column [128, 1] moving, psum accumulates
  [query, feat] over key blocks: 65 moving rows per (key block, query block,
  head) vs 128 in the probs-moving orientation. Denominators land as
  per-query-partition scalars: one reciprocal_approx over [128, 8] and one
  stride-0-broadcast tensor_tensor multiply drain per unit replace the whole
  row-copy/reciprocal/partition-broadcast pipeline of the bf16 baseline.
- The drained attention output [query, feat] is DMA-transposed (128x128
  tiles) into [feat, token] for a bf16 out-projection (baseline o_step).

Cost model: PE ~353k cycles (~147us), Act ~144us (exp only), DVE ~85us
(all psum drains + recip), Pool ~15us (masks). Measured rel err ~1.4e-2
in the numpy model of this exact datapath (gate 2e-2).
"""

import sys

if "/opt/trn_rl_repo" not in sys.path:
    sys.path.insert(0, "/opt/trn_rl_repo")

import ml_dtypes
import numpy as np

import concourse.bass as bass
import concourse.mybir as mybir
from concourse import bacc
from concourse.bass import MemorySpace
from concourse.tile import TileContext

B, T, D = 4, 2048, 1024
H, DH = 16, 64
HG = 8          # heads per core
GW = HG * DH    # group width = 512
P = 128
KD = D // P     # 8 contraction chunks
NTB = T // P    # 16 key blocks of 128
N_CORES = 8
LAG = 6      # kb's between exp issue and AV consumption
PULL = 3     # filler steps pulled forward per kb
WS = 32.0    # host-side weight prescale for fp8 (subnormal avoidance)

F32 = mybir.dt.float32
BF16 = mybir.dt.bfloat16
FP8 = mybir.dt.float8e4
U16 = mybir.dt.uint16
DR = mybir.MatmulPerfMode.DoubleRow


def build_nc():
    nc = bacc.Bacc()

    xs_in = {}
    for nm in ("xq", "xk", "xv"):
        for lv in ("h", "l"):
            xs_in[nm + lv] = nc.dram_tensor(nm + lv, [D, T], FP8, kind="ExternalInput")
    ws_in = {}
    for nm in ("wq", "wk", "wv"):
        for lv in ("h", "l"):
            ws_in[nm + lv] = nc.dram_tensor(nm + lv, [D, GW], FP8, kind="ExternalInput")
    wo = nc.dram_tensor("wo", [GW, D], BF16, kind="ExternalInput")
    msk = nc.dram_tensor("msk", [P, P], BF16, kind="ExternalInput")
    out = nc.dram_tensor("out", [T, D], BF16, kind="ExternalOutput")

    with TileContext(nc) as tc:
        cms = []

        def pool(name, bufs, space=None):
            kw = {"space": space} if space else {}
            cm = tc.tile_pool(name=name, bufs=bufs, **kw)
            cms.append(cm)
            return cm.__enter__()

        big = pool("big", 1)
        ppool = pool("pp", 8)
        xkq = pool("xkq", 4)
        xvp = pool("xvp", 4)
        aop = pool("aop", 2)
        obp = pool("obp", 3)
        sp = pool("sp", 2, MemorySpace.PSUM)    # [128,1024] f32 -> 2 banks x2
        avp = pool("avp", 1, MemorySpace.PSUM)  # av 1 bank + dn 1 bank
        psp = pool("psp", 2, MemorySpace.PSUM)  # [128,512] f32 -> 1 bank x2

        kts = [big.tile([P, T], FP8, name=f"kt{j}") for j in range(4)]
        qts = [big.tile([P, 2, T], FP8, name=f"qt{j}") for j in range(4)]
        vsb = big.tile([P, NTB, HG * 65], BF16, name="vsb")
        wsb = {}
        for nm in ("wq", "wk", "wv"):
            for lv in ("h", "l"):
                wsb[nm + lv] = big.tile([P, KD, GW], FP8, name=f"{nm}{lv}_sb")
        wo_sb = big.tile([P, 4, D], BF16, name="wo_sb")
        mask_sb = big.tile([P, P], BF16, name="mask_sb")
        aotT = [big.tile([P, 4, 512], BF16, name=f"aotT{c}") for c in range(4)]

        vones = vsb.rearrange("p tb (h m) -> p tb h m", h=HG)[:, :, :, 64:65]
        nc.vector.memset(vones.bitcast(U16), 0x3F80)

        lo, hi = slice(0, 64), slice(64, 128)

        xk_t, xq_t, xv_t = {}, {}, {}

        def dma_x(nm, store, r, pool_, tag):
            th = pool_.tile([P, KD, 512], FP8, name=f"x{tag}h", tag=tag)
            tl = pool_.tile([P, KD, 512], FP8, name=f"x{tag}l", tag=tag)
            for t, lv in ((th, "h"), (tl, "l")):
                nc.sync.dma_start(
                    t,
                    xs_in[nm + lv].rearrange("(ko p) t -> p ko t", p=P)[
                        :, :, r * 512:(r + 1) * 512
                    ],
                )
            store[r] = (th, tl)

        def dma_w(nm, lv):
            nc.sync.dma_start(
                wsb[nm + lv],
                ws_in[nm + lv].rearrange("(ko p) j -> p ko j", p=P),
            )

        dma_w("wk", "h")
        dma_w("wk", "l")
        dma_x("xk", xk_t, 0, xkq, "xk")
        dma_w("wq", "h")
        dma_w("wq", "l")
        dma_x("xq", xq_t, 0, xkq, "xq")
        nc.sync.dma_start(mask_sb, msk[:, :])
        dma_w("wv", "h")
        dma_w("wv", "l")
        dma_x("xv", xv_t, 0, xvp, "xv")

        # ---- filler steps: (need, pull, kind, fn) ----
        steps = []

        def kq_step(wh_sb, wl_sb, xst, dst, ch, jb, is_q):
            def fn():
                xht, xlt = xst[ch // 2]
                co = (ch % 2) * 256
                pst = psp.tile([P, GW], F32, name="ps_kq", tag="ps")
                ps = pst[:, 0:256]
                n = 0
                for kdp in range(4):
                    kk = slice(2 * kdp, 2 * kdp + 2)
                    for wsl, xsl in (
                        (wh_sb[:, kk, jb * P:(jb + 1) * P], xht[:, kk, co:co + 256]),
                        (wl_sb[:, kk, jb * P:(jb + 1) * P], xht[:, kk, co:co + 256]),
                        (wh_sb[:, kk, jb * P:(jb + 1) * P], xlt[:, kk, co:co + 256]),
                    ):
                        nc.tensor.matmul(
                            ps, wsl, xsl, start=(n == 0), stop=(n == 11),
                            perf_mode=DR, skip_group_check=True,
                        )
                        n += 1
                cs = slice(ch * 256, (ch + 1) * 256)
                if is_q:
                    nc.vector.tensor_copy(dst[jb][:, 0, cs], ps)
                    nc.vector.scalar_tensor_tensor(
                        out=dst[jb][:, 1, cs], in0=ps, scalar=1.0,
                        in1=dst[jb][:, 0, cs],
                        op0=mybir.AluOpType.mult, op1=mybir.AluOpType.subtract,
                    )
                else:
                    nc.vector.tensor_copy(dst[jb][:, cs], ps)
            return fn

        def v_step(tb):
            def fn():
                xht, xlt = xv_t[tb // 4]
                co = (tb % 4) * P
                ps = psp.tile([P, GW], F32, name="ps_v", tag="ps")
                n = 0
                for kdp in range(4):
                    kk = slice(2 * kdp, 2 * kdp + 2)
                    for xsl, wsl in (
                        (xht[:, kk, co:co + P], wsb["wvh"][:, kk, :]),
                        (xlt[:, kk, co:co + P], wsb["wvh"][:, kk, :]),
                        (xht[:, kk, co:co + P], wsb["wvl"][:, kk, :]),
                    ):
                        nc.tensor.matmul(
                            ps, xsl, wsl, start=(n == 0), stop=(n == 11),
                            perf_mode=DR, skip_group_check=True,
                        )
                        n += 1
                nc.vector.tensor_copy(
                    vsb[:, tb, :].rearrange("p (h m) -> p h m", h=HG)[:, :, 0:64],
                    ps.rearrange("p (h m) -> p h m", h=HG),
                )
            return fn

        def o_step(tb, oc):
            def fn():
                if oc == 0:
                    ob_t[tb % 2] = obp.tile([P, D], BF16, name="ob", tag="ob")
                ob = ob_t[tb % 2]
                c = tb // 4
                ps = psp.tile([P, GW], F32, name="ps_o", tag="ps")
                for jb in range(4):
                    nc.tensor.matmul(
                        ps, aotT[c][:, jb, (tb % 4) * P:(tb % 4 + 1) * P],
                        wo_sb[:, jb, oc * GW:(oc + 1) * GW],
                        start=(jb == 0), stop=(jb == 3),
                    )
                # 1/WS undoes the host-side V-weight prescale
                if tb >= 12:
                    nc.scalar.mul(ob[:, oc * GW:(oc + 1) * GW], ps, 1.0 / WS)
                else:
                    nc.vector.tensor_scalar(
                        out=ob[:, oc * GW:(oc + 1) * GW], in0=ps,
                        scalar1=1.0 / WS, scalar2=None, op0=mybir.AluOpType.mult,
                    )
                nc.sync.dma_start(
                    out[tb * P:(tb + 1) * P, oc * GW:(oc + 1) * GW],
                    ob[:, oc * GW:(oc + 1) * GW],
                )
            return fn

        ob_t = {}

        def wo_dma():
            nc.sync.dma_start(wo_sb, wo.rearrange("(jb p) o -> p jb o", p=P))

        v_idx, o_idx = {}, {}
        for u in range(16):
            r, pr = u // 4, u % 4
            if pr == 1 and r <= 2:
                steps.append((4 * r + 4, u - 1, "x",
                              (lambda r=r: dma_x("xk", xk_t, r + 1, xkq, "xk"))))
                steps.append((4 * r + 4, u - 1, "x",
                              (lambda r=r: dma_x("xq", xq_t, r + 1, xkq, "xq"))))
                steps.append((4 * r + 4, u - 1, "x",
                              (lambda r=r: dma_x("xv", xv_t, r + 1, xvp, "xv"))))
            if u == 3:
                steps.append((u, 0, "x", wo_dma))
            if u == 0:
                for ch in (0, 1):
                    for jb in (0, 1):
                        steps.append((0, 0, "p", kq_step(wsb["wkh"], wsb["wkl"], xk_t, kts, ch, jb, False)))
                for ch in (0, 1):
                    steps.append((0, 0, "p", kq_step(wsb["wkh"], wsb["wkl"], xk_t, kts, ch, 2, False)))
                for ch in (0, 1):
                    steps.append((0, 0, "p", kq_step(wsb["wqh"], wsb["wql"], xq_t, qts, ch, 0, True)))
                for ch in (0, 1):
                    steps.append((1, 0, "p", kq_step(wsb["wkh"], wsb["wkl"], xk_t, kts, ch, 3, False)))
            elif u < 4:
                for ch in (2 * r, 2 * r + 1):
                    steps.append((u, u, "p",
                                  kq_step(wsb["wqh"], wsb["wql"], xq_t, qts, ch, pr, True)))
            else:
                for ch in (2 * r, 2 * r + 1):
                    steps.append((u, u, "p",
                                  kq_step(wsb["wqh"], wsb["wql"], xq_t, qts, ch, pr, True)))
                for ch in (2 * r, 2 * r + 1):
                    steps.append((u, u, "p",
                                  kq_step(wsb["wkh"], wsb["wkl"], xk_t, kts, ch, pr, False)))
            if pr == 0:
                for tb in range(4 * r, 4 * r + 4):
                    v_idx[tb] = len(steps)
                    steps.append((17, u, "p", v_step(tb)))
            if pr in (1, 2, 3) and r >= 1:
                tbs = list(range(4 * (r - 1), 4 * r))
                grp = ({1: tbs[0:1], 2: tbs[1:2], 3: tbs[2:4]}[pr])
                for tb in grp:
                    for oc in (0, 1):
                        steps.append((u, u, "o", o_step(tb, oc)))
        for tb in range(12, 16):
            for oc in (0, 1):
                o_idx[(tb, oc)] = len(steps)
                steps.append((18, 18, "o", o_step(tb, oc)))

        emitted = [False] * len(steps)
        head = [0]

        def emit_step(i):
            if not emitted[i]:
                emitted[i] = True
                steps[i][3]()

        def ensure_v(tb):
            for t in range(tb + 1):
                emit_step(v_idx[t])

        def drain_force(maxneed):
            while head[0] < len(steps) and emitted[head[0]]:
                head[0] += 1
            i = head[0]
            while i < len(steps):
                if not emitted[i] and steps[i][0] <= maxneed:
                    emitted[i] = True
                    steps[i][3]()
                elif not emitted[i] and steps[i][0] > maxneed + 4:
                    break
                i += 1

        def drain_pull(u, limit):
            while head[0] < len(steps) and emitted[head[0]]:
                head[0] += 1
            n, i = 0, head[0]
            scanned = 0
            while i < len(steps) and n < limit and scanned < 80:
                if not emitted[i] and steps[i][1] <= u:
                    emitted[i] = True
                    steps[i][3]()
                    n += 1
                scanned += 1
                i += 1

        # ---- attention units, chunk-major; AV lag queue crosses units ----
        scale = float(DH) ** -0.5 / (WS * WS)
        pend = []       # (unit, kb, F, d0, pp)
        ctx = {}        # unit -> state

        def emit_av(e):
            uu, kb, F, d0, pp = e
            cx = ctx[uu]
            ensure_v(kb)
            if cx["av"] is None:
                cx["av"] = avp.tile([P, 8, 64], F32, name="av", tag="av")
                cx["dn"] = avp.tile([P, 8], F32, name="dn", tag="dn")
            av, dn, c, pr = cx["av"], cx["dn"], cx["c"], cx["pr"]
            j = kb - 4 * c
            ppv = pp.rearrange("p (h q) -> p h q", h=2)
            for qb in range(max(j, 0), 4):
                last = (kb == 4 * c + qb) if cx["diag_last"] else (kb == cx["last"][qb])
                for h in range(2):
                    lhs = ppv[:, h, qb * P - d0:(qb + 1) * P - d0]
                    vcol = (2 * pr + h) * 65
                    nc.tensor.matmul(
                        av[:, 2 * qb + h, :], lhs, vsb[:, kb, vcol:vcol + 64],
                        start=not cx["started"], stop=last, skip_group_check=True,
                    )
                    cx["started"] = True
                    nc.tensor.matmul(
                        dn[:, 2 * qb + h:2 * qb + h + 1], lhs,
                        vsb[:, kb, vcol + 64:vcol + 65],
                        start=not cx["dn_started"], stop=last, skip_group_check=True,
                    )
                    cx["dn_started"] = True
            cx["left"] -= 1
            if cx["diag_last"] and j >= 0:
                emit_tail_qb(cx, j)
            if cx["left"] == 0:
                if not cx["diag_last"]:
                    emit_tail(uu)
                ctx.pop(uu)

        def alloc_tail(cx):
            if cx["rcp"] is None:
                cx["rcp"] = aop.tile([P, 4, 2, 1], F32, name="rcp", tag="rcp")
                cx["ao"] = aop.tile([P, 4, 2, 64], BF16, name="ao", tag="ao")

        def emit_tail_qb(cx, qb):
            # u=15 path: query block qb is final once its diagonal AV lands
            alloc_tail(cx)
            av, dn, rcp, ao = cx["av"], cx["dn"], cx["rcp"], cx["ao"]
            c, pr = cx["c"], cx["pr"]
            nc.vector.reciprocal_approx_fast(
                rcp[:, qb, :, :],
                dn[:, 2 * qb:2 * qb + 2].rearrange("p (n o) -> p n o", o=1),
            )
            nc.vector.tensor_tensor(
                out=ao[:, qb, :, :],
                in0=av[:, 2 * qb:2 * qb + 2, :].rearrange("p (a b) f -> p a b f", b=2)[:, 0],
                in1=rcp[:, qb, :, :].broadcast_to([P, 2, 64]),
                op=mybir.AluOpType.mult,
            )
            nc.sync.dma_start_transpose(
                aotT[c][:, pr, qb * P:(qb + 1) * P],
                ao[:, qb, :, :].rearrange("p a b -> p (a b)"),
            )
            emit_step(o_idx[(12 + qb, 0)])
            emit_step(o_idx[(12 + qb, 1)])

        def emit_tail(uu):
            cx = ctx[uu]
            alloc_tail(cx)
            av, dn, rcp, ao = cx["av"], cx["dn"], cx["rcp"], cx["ao"]
            c, pr = cx["c"], cx["pr"]
            nc.vector.reciprocal_approx_fast(
                rcp.rearrange("p a b o -> p (a b) o"),
                dn.rearrange("p (n o) -> p n o", o=1),
            )
            nc.vector.tensor_tensor(
                out=ao,
                in0=av.rearrange("p (a b) f -> p a b f", b=2),
                in1=rcp.broadcast_to([P, 4, 2, 64]),
                op=mybir.AluOpType.mult,
            )
            for qb in range(4):
                nc.sync.dma_start_transpose(
                    aotT[c][:, pr, qb * P:(qb + 1) * P],
                    ao[:, qb, :, :].rearrange("p a b -> p (a b)"),
                )

        for u in range(16):
            c, pr = u // 4, u % 4
            kt, qt = kts[pr], qts[pr]
            nblk = 4 * (c + 1)
            drain_force(u)

            if u < 15:
                kb_order = list(range(4 * c, nblk)) + list(range(0, 4 * c))
            else:
                kb_order = list(range(nblk))
            last = {}
            for kb in kb_order:
                j = kb - 4 * c
                for qb in range(max(j, 0), 4):
                    last[qb] = kb
            ctx[u] = {"av": None, "dn": None, "rcp": None, "ao": None,
                      "pr": pr, "c": c, "u": u, "left": nblk, "last": last,
                      "started": False, "dn_started": False,
                      "diag_last": u == 15}

            for ki, kb in enumerate(kb_order):
                j = kb - 4 * c
                F = 512 if j < 0 else 512 - 128 * j
                d0 = 0 if j < 0 else 128 * j
                s_pair = sp.tile([P, 1024], F32, name="s_pair", tag="sp")
                ks = slice(kb * P, (kb + 1) * P)
                qs = slice(512 * c + d0, 512 * c + d0 + F)
                kt_lo = kt[lo, ks].rearrange("p (o f) -> p o f", o=1).broadcast_to([64, 2, P])
                kt_hi = kt[hi, ks].rearrange("p (o f) -> p o f", o=1).broadcast_to([64, 2, P])
                nc.tensor.matmul(
                    s_pair[:, 0:F], kt_lo, qt[lo, :, qs], start=True, stop=True,
                    perf_mode=DR, skip_group_check=True,
                )
                nc.tensor.matmul(
                    s_pair[:, 512:512 + F], kt_hi, qt[hi, :, qs],
                    start=True, stop=True, perf_mode=DR, skip_group_check=True,
                )
                pp = ppool.tile([P, 1024], BF16, name="p_pair", tag="pp")
                sv = s_pair.rearrange("p (h q) -> p h q", h=2)[:, :, 0:F]
                pv = pp.rearrange("p (h q) -> p h q", h=2)[:, :, 0:F]
                nc.scalar.activation(
                    pv, sv, mybir.ActivationFunctionType.Exp, scale=scale,
                )
                if j >= 0:
                    # triangular mask on the 128x128 diagonal block (Pool)
                    ppm = pp.rearrange("p (h q) -> p h q", h=2)[:, :, 0:128]
                    nc.gpsimd.tensor_tensor(
                        out=ppm, in0=ppm,
                        in1=mask_sb.rearrange("p (o f) -> p o f", o=1).broadcast_to([P, 2, P]),
                        op=mybir.AluOpType.mult,
                    )
                drain_pull(u, PULL)
                pend.append((u, kb, F, d0, pp))
                lag_u = LAG if u < 15 else 3
                if len(pend) > lag_u:
                    emit_av(pend.pop(0))
                if u == 15 and kb >= 12:
                    for _ in range(2):
                        if pend:
                            emit_av(pend.pop(0))

        while pend:
            emit_av(pend.pop(0))
            drain_pull(16, 1)
        drain_force(99)

        for cm in reversed(cms):
            cm.__exit__(None, None, None)

    nc.finalize()
    return nc


def _split8(a):
    hi = np.asarray(a, np.float32).astype(ml_dtypes.float8_e4m3)
    lo = (np.asarray(a, np.float32) - hi.astype(np.float32)).astype(
        ml_dtypes.float8_e4m3)
    return hi, lo


def make_in_maps(q, k, v, Wq, Wk, Wv, Wo):
    mask_bf = (
        np.arange(P)[None, :] >= np.arange(P)[:, None]
    ).astype(ml_dtypes.bfloat16)
    xs = []
    for b in range(B):
        d = {}
        for nm, t in (("xq", q), ("xk", k), ("xv", v)):
            h, l = _split8(np.ascontiguousarray(t[b].T))
            d[nm + "h"], d[nm + "l"] = h, l
        xs.append(d)
    ws = []
    for g in range(2):
        hs = slice(g * GW, (g + 1) * GW)
        d = {}
        for nm, W in (("wq", Wq), ("wk", Wk), ("wv", Wv)):
            h, l = _split8(np.ascontiguousarray(W[hs, :].T) * WS)
            d[nm + "h"], d[nm + "l"] = h, l
        d["wo"] = np.ascontiguousarray(Wo[:, hs].T).astype(ml_dtypes.bfloat16)
        ws.append(d)
    return [
        {**xs[c // 2], **ws[c % 2], "msk": mask_bf} for c in range(N_CORES)
    ]


_NC_CACHE = None


def kernel(q, k, v, mask, Wq, Wk, Wv, Wo):
    global _NC_CACHE
    if _NC_CACHE is None:
        _NC_CACHE = build_nc()
    nc = _NC_CACHE

    from concourse.bass_utils import run_bass_kernel_spmd

    q, k, v = np.asarray(q), np.asarray(k), np.asarray(v)
    Wq, Wk, Wv, Wo = (np.asarray(t) for t in (Wq, Wk, Wv, Wo))
    in_maps = make_in_maps(q, k, v, Wq, Wk, Wv, Wo)

    r = run_bass_kernel_spmd(nc, in_maps, core_ids=list(range(N_CORES)))
    parts = [np.asarray(r.results[c]["out"], dtype=np.float32) for c in range(N_CORES)]
    y = np.stack([parts[2 * b] + parts[2 * b + 1] for b in range(B)])
    return y
